# revision 3
# baseline (speedup 1.0000x reference)
"""Multi-Head Latent Attention for Trainium2, sharded over 8 NeuronCores.

Sharding: batch (2) x head-groups (4 of 4 heads each) -> 8 cores.

v3 design:
- W_DQ folded into W_UQ / W_QR on the host (weight reparameterization), so
  the device never computes c_Q.
- All matmuls bf16 (fp32 PSUM accumulate); end-to-end max-rel ~4e-3 vs
  float64 (gate 2e-2). Everything SBUF-resident between phases.
- A: q_C^T, q_R^T(rope), c_KV^T, k_rope^T from streamed 512-col x blocks.
  Rope is 3 fused DVE mul-from-psum ops + 1 Pool add (not 6 DVE ops).
- B: k_C^T, v_C from SBUF c_KV^T.
- C: causal attention. Diagonal key-tiles only compute the live query
  columns (N = 512-128r). Softmax denominator: off-diagonal exp tiles are
  pre-summed in groups of 4 on the Pool engine so the PE does 1 ones-matmul
  per group instead of 4. 1/den is broadcast across partitions with a K=1
  PE matmul instead of a DRAM bounce.
- D: W_O projection, bf16 partials to DRAM; host sums groups + transposes.
"""
import numpy as np
import ml_dtypes

import concourse.bass as bass
import concourse.mybir as mybir
import concourse.tile as tile
from concourse import bacc
from concourse.bass_utils import run_bass_kernel_spmd

F32 = mybir.dt.float32
F32R = mybir.dt.float32r
BF16 = mybir.dt.bfloat16
Exp = mybir.ActivationFunctionType.Exp
Mult = mybir.AluOpType.mult
Add = mybir.AluOpType.add
NPBF16 = ml_dtypes.bfloat16

B, S, E = 2, 2048, 2048
H = 16
DH = 128
LOW = 512
R = 64
BASE = 10000.0
HPG = 4               # heads per group (per core)
GCOL = HPG * DH       # 512 columns of this group's heads
P = 128
KE = E // P           # 16 k-tiles over E
KL = LOW // P         # 4 k-tiles over LOW
SBN = S // 512        # 4 seq blocks of 512
NEG = -3.0e38
SCALE = 1.0 / float(np.sqrt(DH + R))

_CACHE = {}


def _rhs_layout(w):
    """[K, N] -> [128, KT, N]: element [p, ko, n] = w[ko*128+p, n].
    Slicing [:, ko, m0:m0+128] is also the lhsT tile for (ko, m-block)."""
    K, N = w.shape
    return np.ascontiguousarray(w.reshape(K // P, P, N).transpose(1, 0, 2))


def _rope_perm_cols(w, rope_dim=R):
    """Permute each rope_dim-column block to [evens, odds] order."""
    K, M = w.shape
    nh = M // rope_dim
    w = w.reshape(K, nh, rope_dim)
    perm = np.concatenate([np.arange(0, rope_dim, 2), np.arange(1, rope_dim, 2)])
    return np.ascontiguousarray(w[:, :, perm].reshape(K, M))


def build_nc():
    nc = bacc.Bacc("TRN2", target_bir_lowering=False, debug=False, num_devices=8)

    xTd = nc.dram_tensor("xT", [P, KE, S], BF16, kind="ExternalInput")
    wDQU = nc.dram_tensor("wDQU", [P, KE, GCOL], BF16, kind="ExternalInput")
    wDQR = nc.dram_tensor("wDQR", [P, KE, HPG * R], BF16, kind="ExternalInput")
    wDKV = nc.dram_tensor("wDKV", [P, KE, LOW], BF16, kind="ExternalInput")
    wKR = nc.dram_tensor("wKR", [P, KE, R], BF16, kind="ExternalInput")
    wUK = nc.dram_tensor("wUK", [P, KL, GCOL], BF16, kind="ExternalInput")
    wUV = nc.dram_tensor("wUV", [P, KL, GCOL], BF16, kind="ExternalInput")
    wO = nc.dram_tensor("wO", [P, HPG, E], BF16, kind="ExternalInput")
    csq = nc.dram_tensor("csq", [P, S], BF16, kind="ExternalInput")
    maskin = nc.dram_tensor("maskin", [P, P], F32, kind="ExternalInput")
    ones_in = nc.dram_tensor("ones_in", [P, 1], BF16, kind="ExternalInput")
    onescol_in = nc.dram_tensor("onescol_in", [1, P], F32R, kind="ExternalInput")

    outT = nc.dram_tensor("outT", [E, S], BF16, kind="ExternalOutput")

    with tile.TileContext(nc) as tc:
        with tc.tile_pool(name="persist", bufs=1) as persist:
            t_cs = persist.tile([P, S], BF16, tag="cs")
            t_mask = persist.tile([P, P], F32, tag="mask")
            t_ones = persist.tile([P, 1], BF16, tag="ones")
            t_onescol = persist.tile([1, P], F32R, tag="onescol")
            qCT = persist.tile([P, HPG, S], BF16, tag="qCT")
            qrT = persist.tile([R, HPG, S], BF16, tag="qrT")
            kropeT = persist.tile([R, S], BF16, tag="kropeT")
            kCT = persist.tile([P, HPG, S], BF16, tag="kCT")
            vC = persist.tile([P, S // P, GCOL], BF16, tag="vC")
            ckvT = persist.tile([P, KL, S], BF16, tag="ckvT")
            t_wuk = persist.tile([P, KL, GCOL], BF16, tag="wuk")
            t_wuv = persist.tile([P, KL, GCOL], BF16, tag="wuv")
            t_wo = persist.tile([P, HPG, E], BF16, tag="wo")

            # ---- Phase A: q_C^T, q_R^T(rope), c_KV^T, k_rope^T from x ----
            with (
                tc.tile_pool(name="aw", bufs=1) as awp,
                tc.tile_pool(name="xp", bufs=2) as xp,
                tc.tile_pool(name="ropew", bufs=1) as rp,
                tc.tile_pool(name="ps_a", bufs=3, space="PSUM") as ps_a,
                tc.tile_pool(name="ps_kr", bufs=1, space="PSUM") as ps_kr,
            ):
                t_wdqu = awp.tile([P, KE, GCOL], BF16, tag="wdqu")
                xt0 = xp.tile([P, KE, 512], BF16, tag="xt")
                nc.sync.dma_start(out=t_wdqu[:, 0:8, :], in_=wDQU[:, 0:8, :])
                nc.sync.dma_start(out=xt0[:, 0:8, :], in_=xTd[:, 0:8, 0:512])
                nc.sync.dma_start(out=t_wdqu[:, 8:KE, :], in_=wDQU[:, 8:KE, :])
                nc.sync.dma_start(out=xt0[:, 8:KE, :], in_=xTd[:, 8:KE, 0:512])
                t_wdqr = awp.tile([P, KE, HPG * R], BF16, tag="wdqr")
                nc.sync.dma_start(out=t_wdqr, in_=wDQR[:, :, :])
                nc.sync.dma_start(out=t_cs, in_=csq[:, :])
                t_wdkv = awp.tile([P, KE, LOW], BF16, tag="wdkv")
                nc.sync.dma_start(out=t_wdkv, in_=wDKV[:, :, :])
                t_wkr = awp.tile([P, KE, R], BF16, tag="wkr")
                nc.sync.dma_start(out=t_wkr, in_=wKR[:, :, :])
                nc.sync.dma_start(out=t_ones, in_=ones_in[:, :])
                nc.sync.dma_start(out=t_onescol, in_=onescol_in[:, :])
                nc.sync.dma_start(out=t_mask, in_=maskin[:, :])

                def rope_write(psum, base, dst, ssl):
                    """dst[0:64,:] = rope(psum[base:base+64, :]), 512 wide.
                    t1 = [x1;x2]*[c;c] on DVE (fused mul from psum),
                    t2 = [x2;x1]*[-s;s] on DVE (two 32-row fused muls),
                    dst = t1 + t2 on Pool."""
                    t1 = rp.tile([R, 512], F32, tag="t1")
                    nc.vector.tensor_tensor(t1, psum[base:base + R, :],
                                            t_cs[0:R, ssl], Mult)
                    t2 = rp.tile([R, 512], F32, tag="t2")
                    nc.vector.tensor_tensor(t2[0:32, :],
                                            psum[base + 32:base + R, :],
                                            t_cs[R:R + 32, ssl], Mult)
                    nc.vector.tensor_tensor(t2[32:R, :],
                                            psum[base:base + 32, :],
                                            t_cs[R + 32:2 * R, ssl], Mult)
                    nc.gpsimd.tensor_tensor(dst, t1, t2, Add)

                for sb in range(SBN):
                    ssl = slice(sb * 512, (sb + 1) * 512)
                    if sb == 0:
                        xt = xt0
                    else:
                        xt = xp.tile([P, KE, 512], BF16, tag="xt")
                        nc.sync.dma_start(out=xt, in_=xTd[:, :, ssl])
                    for mo in range(HPG):      # q_C^T
                        psum = ps_a.tile([P, 512], F32, tag="pa")
                        for k in range(KE):
                            nc.tensor.matmul(
                                psum, t_wdqu[:, k, mo * P:(mo + 1) * P],
                                xt[:, k, :], start=(k == 0), stop=(k == KE - 1))
                        nc.vector.tensor_copy(out=qCT[:, mo, ssl], in_=psum)
                    for j in range(2):         # q_R^T (2 heads per psum)
                        psum = ps_a.tile([P, 512], F32, tag="pa")
                        for k in range(KE):
                            nc.tensor.matmul(
                                psum, t_wdqr[:, k, j * P:(j + 1) * P],
                                xt[:, k, :], start=(k == 0), stop=(k == KE - 1))
                        rope_write(psum, 0, qrT[:, 2 * j, ssl], ssl)
                        rope_write(psum, R, qrT[:, 2 * j + 1, ssl], ssl)
                    for mo in range(KL):       # c_KV^T
                        psum = ps_a.tile([P, 512], F32, tag="pa")
                        for k in range(KE):
                            nc.tensor.matmul(
                                psum, t_wdkv[:, k, mo * P:(mo + 1) * P],
                                xt[:, k, :], start=(k == 0), stop=(k == KE - 1))
                        nc.vector.tensor_copy(out=ckvT[:, mo, ssl], in_=psum)
                    psum = ps_kr.tile([R, 512], F32, tag="pkr")   # k_rope^T
                    for k in range(KE):
                        nc.tensor.matmul(psum, t_wkr[:, k, :], xt[:, k, :],
                                         start=(k == 0), stop=(k == KE - 1))
                    rope_write(psum, 0, kropeT[:, ssl], ssl)

            # ---- Phase B: k_C^T, v_C from c_KV^T ------------------------
            with (
                tc.tile_pool(name="ps_kc", bufs=2, space="PSUM") as ps_kc,
                tc.tile_pool(name="ps_vc", bufs=2, space="PSUM") as ps_vc,
            ):
                nc.sync.dma_start(out=t_wuk, in_=wUK[:, :, :])
                nc.sync.dma_start(out=t_wuv, in_=wUV[:, :, :])
                nc.sync.dma_start(out=t_wo, in_=wO[:, :, :])
                for sb in range(SBN):
                    ssl = slice(sb * 512, (sb + 1) * 512)
                    for h in range(HPG):
                        psum = ps_kc.tile([P, 512], F32, tag="p")
                        for k in range(KL):
                            nc.tensor.matmul(
                                psum, t_wuk[:, k, h * P:(h + 1) * P],
                                ckvT[:, k, ssl], start=(k == 0),
                                stop=(k == KL - 1))
                        nc.vector.tensor_copy(out=kCT[:, h, ssl], in_=psum)
                    for loc in range(4):
                        st = sb * 4 + loc
                        psum = ps_vc.tile([P, GCOL], F32, tag="p")
                        for k in range(KL):
                            nc.tensor.matmul(
                                psum, ckvT[:, k, st * P:(st + 1) * P],
                                t_wuv[:, k, :], start=(k == 0),
                                stop=(k == KL - 1))
                        nc.vector.tensor_copy(out=vC[:, st, :], in_=psum)

            # ---- Phase C: attention;  Phase D: W_O ----------------------
            with (
                tc.tile_pool(name="att", bufs=8) as att,
                tc.tile_pool(name="accp", bufs=2) as accp,
                tc.tile_pool(name="bcp", bufs=2) as bcp,
                tc.tile_pool(name="rcp", bufs=2) as rcp,
                tc.tile_pool(name="aop", bufs=2) as aop,
                tc.tile_pool(name="oout", bufs=3) as oout,
                tc.tile_pool(name="ps_s", bufs=4, space="PSUM") as ps_s,
                tc.tile_pool(name="ps_o", bufs=2, space="PSUM") as ps_o,
                tc.tile_pool(name="ps_d", bufs=1, space="PSUM") as ps_d,
                tc.tile_pool(name="ps_bc", bufs=1, space="PSUM") as ps_bc,
            ):
                def norm_head(aoT, h, psum_o, recip):
                    """Deferred 1/den broadcast + aoT[h] = psum_o * bc."""
                    psum_b = ps_bc.tile([P, 512], F32, tag="p")
                    nc.tensor.matmul(psum_b, t_onescol, recip,
                                     start=True, stop=True)
                    bc = bcp.tile([P, 512], BF16, tag="bc")
                    nc.vector.tensor_copy(out=bc, in_=psum_b)
                    nc.vector.tensor_tensor(aoT[:, h, :], psum_o, bc, Mult)

                for sb in range(SBN):
                    ssl = slice(sb * 512, (sb + 1) * 512)
                    aoT = aop.tile([P, HPG, 512], BF16, tag="aoT")
                    T = 4 * (sb + 1)
                    deferred = None
                    for h in range(HPG):
                        psum_o = ps_o.tile([P, 512], F32, tag="p")
                        psum_d = ps_d.tile([1, 512], F32, tag="p")
                        pend_den = None     # (acc, start) awaiting PE issue
                        # off-diagonal key tiles, denominator batched by 4
                        for g in range(sb):
                            exps = []
                            for j in range(4):
                                tt = 4 * g + j
                                tsl = slice(tt * P, (tt + 1) * P)
                                psum_s = ps_s.tile([P, 512], F32, tag="p")
                                nc.tensor.matmul(psum_s, kCT[:, h, tsl],
                                                 qCT[:, h, ssl],
                                                 start=True, stop=False)
                                nc.tensor.matmul(psum_s, kropeT[:, tsl],
                                                 qrT[:, h, ssl],
                                                 start=False, stop=True)
                                expT = att.tile([P, 512], BF16, tag="expT")
                                nc.scalar.activation(out=expT, in_=psum_s,
                                                     func=Exp)
                                nc.tensor.matmul(
                                    psum_o, vC[:, tt, h * DH:(h + 1) * DH],
                                    expT, start=(tt == 0), stop=False)
                                exps.append(expT)
                            if pend_den is not None:
                                nc.tensor.matmul(psum_d, t_ones, pend_den[0],
                                                 start=pend_den[1], stop=False)
                            acc = accp.tile([P, 512], BF16, tag="acc")
                            nc.vector.tensor_tensor(acc, exps[0], exps[1], Add)
                            nc.vector.tensor_tensor(acc, acc, exps[2], Add)
                            nc.vector.tensor_tensor(acc, acc, exps[3], Add)
                            pend_den = (acc, g == 0)
                            if g == 0 and deferred is not None:
                                norm_head(*deferred)
                                deferred = None
                        # diagonal key tiles, trimmed to live queries
                        for r in range(4):
                            tt = 4 * sb + r
                            tsl = slice(tt * P, (tt + 1) * P)
                            npr = 512 - P * r
                            qsl = slice(sb * 512 + P * r, (sb + 1) * 512)
                            psum_s = ps_s.tile([P, 512], F32, tag="p")
                            nc.tensor.matmul(psum_s[:, 0:npr],
                                             kCT[:, h, tsl], qCT[:, h, qsl],
                                             start=True, stop=False)
                            nc.tensor.matmul(psum_s[:, 0:npr],
                                             kropeT[:, tsl], qrT[:, h, qsl],
                                             start=False, stop=True)
                            nc.vector.tensor_tensor(psum_s[:, 0:P],
                                                    psum_s[:, 0:P],
                                                    t_mask, Add)
                            expT = att.tile([P, 512], BF16, tag="expT")
                            nc.scalar.activation(out=expT[:, 0:npr],
                                                 in_=psum_s[:, 0:npr],
                                                 func=Exp)
                            first = (sb == 0 and r == 0)
                            nc.tensor.matmul(
                                psum_o[:, P * r:512],
                                vC[:, tt, h * DH:(h + 1) * DH],
                                expT[:, 0:npr], start=first,
                                stop=(r == 3))
                            if r == 0 and pend_den is not None:
                                nc.tensor.matmul(psum_d, t_ones, pend_den[0],
                                                 start=pend_den[1], stop=False)
                                pend_den = None
                            nc.tensor.matmul(
                                psum_d[0:1, P * r:512], t_ones,
                                expT[:, 0:npr], start=first,
                                stop=(r == 3))
                            if r == 1 and deferred is not None:
                                norm_head(*deferred)
                                deferred = None
                        recip = rcp.tile([1, 512], F32R, tag="recip")
                        with nc.allow_low_precision(
                                reason="f32r is 32-bit storage; fp32 recip"):
                            nc.vector.reciprocal(out=recip, in_=psum_d)
                        deferred = (aoT, h, psum_o, recip)
                    norm_head(*deferred)
                    deferred = None
                    for mo in range(KE):
                        psum_w = ps_s.tile([P, 512], F32, tag="p")
                        for k in range(HPG):
                            nc.tensor.matmul(psum_w,
                                             t_wo[:, k, mo * P:(mo + 1) * P],
                                             aoT[:, k, :],
                                             start=(k == 0), stop=(k == HPG - 1))
                        ot = oout.tile([P, 512], BF16, tag="oout")
                        nc.vector.tensor_copy(out=ot, in_=psum_w)
                        nc.sync.dma_start(out=outT[mo * P:(mo + 1) * P, ssl],
                                          in_=ot)

    nc.compile()
    return nc


def _host_inputs(inputs):
    """Per-core input maps (host-side sharding + weight pre-tiling)."""
    x = inputs["x"]
    W_DQ = inputs["W_DQ"].astype(np.float32)
    W_UQ = inputs["W_UQ"].astype(np.float32)
    W_QR = inputs["W_QR"].astype(np.float32)
    W_DKV = inputs["W_DKV"].astype(np.float32)
    W_UK = inputs["W_UK"].astype(np.float32)
    W_KR = inputs["W_KR"].astype(np.float32)
    W_UV = inputs["W_UV"].astype(np.float32)
    W_O = inputs["W_O"].astype(np.float32)

    # fold W_DQ into the query up-projections (pure reparameterization)
    W_DQU = (W_DQ @ W_UQ) * SCALE                  # [E, E]
    W_DQR = (W_DQ @ W_QR) * SCALE                  # [E, R*H]

    wDKV_t = _rhs_layout(W_DKV).astype(NPBF16)
    wKR_t = _rhs_layout(_rope_perm_cols(W_KR)).astype(NPBF16)

    half = R // 2
    freqs = BASE ** (-np.arange(half, dtype=np.float64) / half)
    theta = np.arange(S, dtype=np.float64)[None, :] * freqs[:, None]   # [32, S]
    cs = np.concatenate([np.cos(theta), np.cos(theta),
                         -np.sin(theta), np.sin(theta)], 0).astype(NPBF16)
    p = np.arange(P)[:, None]
    f = np.arange(P)[None, :]
    maskadd = np.where(p <= f, 0.0, NEG).astype(np.float32)
    ones = np.ones((P, 1), NPBF16)
    onescol = np.ones((1, P), np.float32)

    shared = {
        "wDKV": wDKV_t, "wKR": wKR_t, "csq": cs, "maskin": maskadd,
        "ones_in": ones, "onescol_in": onescol,
    }
    gsets = []
    for g in range(4):
        cs0, ce0 = g * GCOL, (g + 1) * GCOL
        gsets.append({
            "wDQU": _rhs_layout(W_DQU[:, cs0:ce0]).astype(NPBF16),
            "wDQR": _rhs_layout(_rope_perm_cols(
                W_DQR[:, g * HPG * R:(g + 1) * HPG * R])).astype(NPBF16),
            "wUK": _rhs_layout(W_UK[:, cs0:ce0]).astype(NPBF16),
            "wUV": _rhs_layout(W_UV[:, cs0:ce0]).astype(NPBF16),
            "wO": _rhs_layout(W_O[cs0:ce0, :]).astype(NPBF16),
        })
    in_maps = []
    for c in range(8):
        b, g = divmod(c, 4)
        xT = np.ascontiguousarray(
            x[b].T.reshape(KE, P, S).transpose(1, 0, 2)).astype(NPBF16)
        m = {"xT": xT}
        m.update(shared)
        m.update(gsets[g])
        in_maps.append(m)
    return in_maps


def _assemble(results):
    out = np.empty((B, S, E), np.float32)
    for b in range(B):
        acc = results[4 * b]["outT"].astype(np.float32)
        for g in range(1, 4):
            acc = acc + results[4 * b + g]["outT"].astype(np.float32)
        out[b] = acc.T
    return out


def kernel(**inputs):
    inputs = {k: np.asarray(v) for k, v in inputs.items()}
    if "nc" not in _CACHE:
        _CACHE["nc"] = build_nc()
    nc = _CACHE["nc"]
    in_maps = _host_inputs(inputs)
    res = run_bass_kernel_spmd(nc, in_maps, core_ids=list(range(8)))
    return _assemble(res.results)


# revision 4
# speedup vs baseline: 1.0875x; 1.0875x over previous
"""Multi-Head Latent Attention for Trainium2, sharded over 8 NeuronCores.

Sharding: batch (2) x head-groups (4 of 4 heads each) -> 8 cores.

v3 design:
- W_DQ folded into W_UQ / W_QR on the host (weight reparameterization), so
  the device never computes c_Q.
- All matmuls bf16 (fp32 PSUM accumulate); end-to-end max-rel ~4e-3 vs
  float64 (gate 2e-2). Everything SBUF-resident between phases.
- A: q_C^T, q_R^T(rope), c_KV^T, k_rope^T from streamed 512-col x blocks.
  Rope is 3 fused DVE mul-from-psum ops + 1 Pool add (not 6 DVE ops).
- B: k_C^T, v_C from SBUF c_KV^T.
- C: causal attention. Diagonal key-tiles only compute the live query
  columns (N = 512-128r). Softmax denominator: off-diagonal exp tiles are
  pre-summed in groups of 4 on the Pool engine so the PE does 1 ones-matmul
  per group instead of 4. 1/den is broadcast across partitions with a K=1
  PE matmul instead of a DRAM bounce.
- D: W_O projection, bf16 partials to DRAM; host sums groups + transposes.
"""
import numpy as np
import ml_dtypes

import concourse.bass as bass
import concourse.mybir as mybir
import concourse.tile as tile
from concourse import bacc
from concourse.bass_utils import run_bass_kernel_spmd

F32 = mybir.dt.float32
F32R = mybir.dt.float32r
BF16 = mybir.dt.bfloat16
Exp = mybir.ActivationFunctionType.Exp
Mult = mybir.AluOpType.mult
Add = mybir.AluOpType.add
NPBF16 = ml_dtypes.bfloat16

B, S, E = 2, 2048, 2048
H = 16
DH = 128
LOW = 512
R = 64
BASE = 10000.0
HPG = 4               # heads per group (per core)
GCOL = HPG * DH       # 512 columns of this group's heads
P = 128
KE = E // P           # 16 k-tiles over E
KL = LOW // P         # 4 k-tiles over LOW
SBN = S // 512        # 4 seq blocks of 512
NEG = -3.0e38
SCALE = 1.0 / float(np.sqrt(DH + R))

_CACHE = {}


def _rhs_layout(w):
    """[K, N] -> [128, KT, N]: element [p, ko, n] = w[ko*128+p, n].
    Slicing [:, ko, m0:m0+128] is also the lhsT tile for (ko, m-block)."""
    K, N = w.shape
    return np.ascontiguousarray(w.reshape(K // P, P, N).transpose(1, 0, 2))


def _rope_perm_cols(w, rope_dim=R):
    """Permute each rope_dim-column block to [evens, odds] order."""
    K, M = w.shape
    nh = M // rope_dim
    w = w.reshape(K, nh, rope_dim)
    perm = np.concatenate([np.arange(0, rope_dim, 2), np.arange(1, rope_dim, 2)])
    return np.ascontiguousarray(w[:, :, perm].reshape(K, M))


def build_nc():
    nc = bacc.Bacc("TRN2", target_bir_lowering=False, debug=False, num_devices=8)

    xTd = nc.dram_tensor("xT", [P, KE, S], BF16, kind="ExternalInput")
    wDQU = nc.dram_tensor("wDQU", [P, KE, GCOL], BF16, kind="ExternalInput")
    wDQR = nc.dram_tensor("wDQR", [P, KE, HPG * R], BF16, kind="ExternalInput")
    wDKV = nc.dram_tensor("wDKV", [P, KE, LOW], BF16, kind="ExternalInput")
    wKR = nc.dram_tensor("wKR", [P, KE, R], BF16, kind="ExternalInput")
    wUK = nc.dram_tensor("wUK", [P, KL, GCOL], BF16, kind="ExternalInput")
    wUV = nc.dram_tensor("wUV", [P, KL, GCOL], BF16, kind="ExternalInput")
    wO = nc.dram_tensor("wO", [P, HPG, E], BF16, kind="ExternalInput")
    csq = nc.dram_tensor("csq", [P, S], BF16, kind="ExternalInput")
    maskin = nc.dram_tensor("maskin", [P, P], BF16, kind="ExternalInput")
    ones_in = nc.dram_tensor("ones_in", [P, 1], BF16, kind="ExternalInput")
    onescol_in = nc.dram_tensor("onescol_in", [1, P], F32R, kind="ExternalInput")

    outT = nc.dram_tensor("outT", [E, S], BF16, kind="ExternalOutput")

    with tile.TileContext(nc) as tc:
        with tc.tile_pool(name="persist", bufs=1) as persist:
            t_cs = persist.tile([P, S], BF16, tag="cs")
            t_mask = persist.tile([P, P], BF16, tag="mask")
            t_ones = persist.tile([P, 1], BF16, tag="ones")
            t_onescol = persist.tile([1, P], F32R, tag="onescol")
            qCT = persist.tile([P, HPG, S], BF16, tag="qCT")
            qrT = persist.tile([R, HPG, S], BF16, tag="qrT")
            kropeT = persist.tile([R, S], BF16, tag="kropeT")
            kCT = persist.tile([P, HPG, S], BF16, tag="kCT")
            vC = persist.tile([P, S // P, GCOL], BF16, tag="vC")
            ckvT = persist.tile([P, KL, S], BF16, tag="ckvT")
            t_wuk = persist.tile([P, KL, GCOL], BF16, tag="wuk")
            t_wuv = persist.tile([P, KL, GCOL], BF16, tag="wuv")
            t_wo = persist.tile([P, HPG, E], BF16, tag="wo")

            # one [128,512] fp32 psum pool spans all phases: A projections,
            # B up-projections, C scores + bcast, D W_O. Sharing the pool
            # (same tag) avoids bank-reuse barriers at phase boundaries.
            import contextlib
            stack = contextlib.ExitStack()
            ps = stack.enter_context(tc.tile_pool(name="ps", bufs=4,
                                                  space="PSUM"))

            # ---- Phase A: q_C^T, q_R^T(rope), c_KV^T, k_rope^T from x ----
            with (
                tc.tile_pool(name="aw", bufs=1) as awp,
                tc.tile_pool(name="xp", bufs=2) as xp,
                tc.tile_pool(name="ropew", bufs=1) as rp,
                tc.tile_pool(name="ps_kr", bufs=1, space="PSUM") as ps_kr,
            ):
                nc.sync.dma_start(out=t_ones, in_=ones_in[:, :])
                t_wdqu = awp.tile([P, KE, GCOL], BF16, tag="wdqu")
                xt0 = xp.tile([P, KE, 512], BF16, tag="xt")
                for q0 in range(0, KE, 4):
                    q1 = q0 + 4
                    nc.sync.dma_start(out=t_wdqu[:, q0:q1, :],
                                      in_=wDQU[:, q0:q1, :])
                    nc.sync.dma_start(out=xt0[:, q0:q1, :],
                                      in_=xTd[:, q0:q1, 0:512])
                warm = awp.tile([P, 1], BF16, tag="warm")
                nc.scalar.activation(out=warm, in_=t_ones, func=Exp)
                t_wdqr = awp.tile([P, KE, HPG * R], BF16, tag="wdqr")
                nc.sync.dma_start(out=t_wdqr, in_=wDQR[:, :, :])
                nc.sync.dma_start(out=t_cs, in_=csq[:, :])
                t_wdkv = awp.tile([P, KE, LOW], BF16, tag="wdkv")
                nc.sync.dma_start(out=t_wdkv, in_=wDKV[:, :, :])
                t_wkr = awp.tile([P, KE, R], BF16, tag="wkr")
                nc.sync.dma_start(out=t_wkr, in_=wKR[:, :, :])
                nc.sync.dma_start(out=t_onescol, in_=onescol_in[:, :])
                nc.sync.dma_start(out=t_mask, in_=maskin[:, :])

                def rope_write(psum, base, dst, ssl):
                    """dst[0:64,:] = rope(psum[base:base+64, :]), 512 wide.
                    t1 = [x1;x2]*[c;c] on DVE (fused mul from psum),
                    t2 = [x2;x1]*[-s;s] on DVE (two 32-row fused muls),
                    dst = t1 + t2 on Pool."""
                    t1 = rp.tile([R, 512], F32, tag="t1")
                    nc.vector.tensor_tensor(t1, psum[base:base + R, :],
                                            t_cs[0:R, ssl], Mult)
                    t2 = rp.tile([R, 512], F32, tag="t2")
                    nc.vector.tensor_tensor(t2[0:32, :],
                                            psum[base + 32:base + R, :],
                                            t_cs[R:R + 32, ssl], Mult)
                    nc.vector.tensor_tensor(t2[32:R, :],
                                            psum[base:base + 32, :],
                                            t_cs[R + 32:2 * R, ssl], Mult)
                    nc.gpsimd.tensor_tensor(dst, t1, t2, Add)

                for sb in range(SBN):
                    ssl = slice(sb * 512, (sb + 1) * 512)
                    if sb == 0:
                        xt = xt0
                    else:
                        xt = xp.tile([P, KE, 512], BF16, tag="xt")
                        nc.sync.dma_start(out=xt, in_=xTd[:, :, ssl])
                    for mo in range(HPG):      # q_C^T
                        psum = ps.tile([P, 512], F32, tag="p")
                        for k in range(KE):
                            nc.tensor.matmul(
                                psum, t_wdqu[:, k, mo * P:(mo + 1) * P],
                                xt[:, k, :], start=(k == 0), stop=(k == KE - 1))
                        nc.vector.tensor_copy(out=qCT[:, mo, ssl], in_=psum)
                    for j in range(2):         # q_R^T (2 heads per psum)
                        psum = ps.tile([P, 512], F32, tag="p")
                        for k in range(KE):
                            nc.tensor.matmul(
                                psum, t_wdqr[:, k, j * P:(j + 1) * P],
                                xt[:, k, :], start=(k == 0), stop=(k == KE - 1))
                        rope_write(psum, 0, qrT[:, 2 * j, ssl], ssl)
                        rope_write(psum, R, qrT[:, 2 * j + 1, ssl], ssl)
                    for mo in range(KL):       # c_KV^T
                        psum = ps.tile([P, 512], F32, tag="p")
                        for k in range(KE):
                            nc.tensor.matmul(
                                psum, t_wdkv[:, k, mo * P:(mo + 1) * P],
                                xt[:, k, :], start=(k == 0), stop=(k == KE - 1))
                        nc.vector.tensor_copy(out=ckvT[:, mo, ssl], in_=psum)
                    psum = ps_kr.tile([R, 512], F32, tag="pkr")   # k_rope^T
                    for k in range(KE):
                        nc.tensor.matmul(psum, t_wkr[:, k, :], xt[:, k, :],
                                         start=(k == 0), stop=(k == KE - 1))
                    rope_write(psum, 0, kropeT[:, ssl], ssl)

            # ---- Phase B: k_C^T, v_C from c_KV^T ------------------------
            with (
                tc.tile_pool(name="ps_o", bufs=2, space="PSUM") as ps_o,
                tc.tile_pool(name="ps_d", bufs=2, space="PSUM") as ps_d,
            ):
                nc.sync.dma_start(out=t_wuk, in_=wUK[:, :, :])
                nc.sync.dma_start(out=t_wuv, in_=wUV[:, :, :])
                nc.sync.dma_start(out=t_wo, in_=wO[:, :, :])
                for sb in range(SBN):
                    ssl = slice(sb * 512, (sb + 1) * 512)
                    for h in range(HPG):
                        psum = ps.tile([P, 512], F32, tag="p")
                        for k in range(KL):
                            nc.tensor.matmul(
                                psum, t_wuk[:, k, h * P:(h + 1) * P],
                                ckvT[:, k, ssl], start=(k == 0),
                                stop=(k == KL - 1))
                        nc.vector.tensor_copy(out=kCT[:, h, ssl], in_=psum)
                    for loc in range(4):
                        st = sb * 4 + loc
                        psum = ps.tile([P, GCOL], F32, tag="p")
                        for k in range(KL):
                            nc.tensor.matmul(
                                psum, ckvT[:, k, st * P:(st + 1) * P],
                                t_wuv[:, k, :], start=(k == 0),
                                stop=(k == KL - 1))
                        nc.vector.tensor_copy(out=vC[:, st, :], in_=psum)

                # ---- Phase C: attention;  Phase D: W_O ------------------
                att = stack.enter_context(tc.tile_pool(name="att", bufs=8))
                accp = stack.enter_context(tc.tile_pool(name="accp", bufs=2))
                bcp = stack.enter_context(tc.tile_pool(name="bcp", bufs=2))
                rcp = stack.enter_context(tc.tile_pool(name="rcp", bufs=2))
                aop = stack.enter_context(tc.tile_pool(name="aop", bufs=2))
                oout = stack.enter_context(tc.tile_pool(name="oout", bufs=8))
                def norm_head(aoT, h, aou, recip):
                    """Deferred 1/den broadcast + aoT[h] = aou * bc."""
                    psum_b = ps.tile([P, 512], F32, tag="p")
                    nc.tensor.matmul(psum_b, t_onescol, recip,
                                     start=True, stop=True)
                    nc.vector.tensor_tensor(aoT[:, h, :], aou, psum_b, Mult)

                for sb in range(SBN):
                    ssl = slice(sb * 512, (sb + 1) * 512)
                    aoT = aop.tile([P, HPG, 512], BF16, tag="aoT")
                    T = 4 * (sb + 1)
                    deferred = None
                    for h in range(HPG):
                        psum_o = ps_o.tile([P, 512], F32, tag="p")
                        psum_d = ps_d.tile([1, 512], F32, tag="p")
                        pend_den = None     # (acc, start) awaiting PE issue
                        # off-diagonal key tiles, denominator batched by 4
                        for g in range(sb):
                            exps = []
                            for j in range(4):
                                tt = 4 * g + j
                                tsl = slice(tt * P, (tt + 1) * P)
                                psum_s = ps.tile([P, 512], F32, tag="p")
                                nc.tensor.matmul(psum_s, kCT[:, h, tsl],
                                                 qCT[:, h, ssl],
                                                 start=True, stop=False)
                                nc.tensor.matmul(psum_s, kropeT[:, tsl],
                                                 qrT[:, h, ssl],
                                                 start=False, stop=True)
                                expT = att.tile([P, 512], BF16, tag="expT")
                                nc.scalar.activation(out=expT, in_=psum_s,
                                                     func=Exp)
                                nc.tensor.matmul(
                                    psum_o, vC[:, tt, h * DH:(h + 1) * DH],
                                    expT, start=(tt == 0), stop=False)
                                exps.append(expT)
                            if pend_den is not None:
                                nc.tensor.matmul(psum_d, t_ones, pend_den[0],
                                                 start=pend_den[1], stop=False)
                            acc = accp.tile([P, 512], BF16, tag="acc")
                            nc.vector.tensor_tensor(acc, exps[0], exps[1], Add)
                            nc.vector.tensor_tensor(acc, acc, exps[2], Add)
                            nc.vector.tensor_tensor(acc, acc, exps[3], Add)
                            pend_den = (acc, g == 0)
                            if g == 0 and deferred is not None:
                                norm_head(*deferred)
                                deferred = None
                        # diagonal key tiles, trimmed to live queries
                        for r in range(4):
                            tt = 4 * sb + r
                            tsl = slice(tt * P, (tt + 1) * P)
                            npr = 512 - P * r
                            qsl = slice(sb * 512 + P * r, (sb + 1) * 512)
                            psum_s = ps.tile([P, 512], F32, tag="p")
                            nc.tensor.matmul(psum_s[:, 0:npr],
                                             kCT[:, h, tsl], qCT[:, h, qsl],
                                             start=True, stop=False)
                            nc.tensor.matmul(psum_s[:, 0:npr],
                                             kropeT[:, tsl], qrT[:, h, qsl],
                                             start=False, stop=True)
                            expT = att.tile([P, 512], BF16, tag="expT")
                            nc.scalar.activation(out=expT[:, 0:npr],
                                                 in_=psum_s[:, 0:npr],
                                                 func=Exp)
                            nc.gpsimd.tensor_tensor(expT[:, 0:P],
                                                    expT[:, 0:P],
                                                    t_mask, Mult)
                            first = (sb == 0 and r == 0)
                            nc.tensor.matmul(
                                psum_o[:, P * r:512],
                                vC[:, tt, h * DH:(h + 1) * DH],
                                expT[:, 0:npr], start=first,
                                stop=(r == 3))
                            if r == 0 and pend_den is not None:
                                nc.tensor.matmul(psum_d, t_ones, pend_den[0],
                                                 start=pend_den[1], stop=False)
                                pend_den = None
                            nc.tensor.matmul(
                                psum_d[0:1, P * r:512], t_ones,
                                expT[:, 0:npr], start=first,
                                stop=(r == 3))
                            if r == 1 and deferred is not None:
                                norm_head(*deferred)
                                deferred = None
                        aou = bcp.tile([P, 512], BF16, tag="aou")
                        nc.scalar.activation(
                            out=aou, in_=psum_o,
                            func=mybir.ActivationFunctionType.Copy)
                        recip = rcp.tile([1, 512], F32R, tag="recip")
                        with nc.allow_low_precision(
                                reason="f32r is 32-bit storage; fp32 recip"):
                            nc.vector.reciprocal(out=recip, in_=psum_d)
                        deferred = (aoT, h, aou, recip)
                    # W_O: contract heads 0..2 immediately; defer each
                    # group's final h3 step one group so the last head's
                    # normalization chain is hidden behind real work.
                    pend_w = []         # [(psum_w, mo)], flushed 2 behind
                    def fin_w(psum_w, mo):
                        nc.tensor.matmul(psum_w,
                                         t_wo[:, 3, mo * P:(mo + 1) * P],
                                         aoT[:, 3, :], start=False, stop=True)
                        ot = oout.tile([P, 512], BF16, tag="oout")
                        if mo % 2 == 0:
                            nc.vector.tensor_copy(out=ot, in_=psum_w)
                        else:
                            nc.scalar.activation(out=ot, in_=psum_w,
                                                 func=mybir.ActivationFunctionType.Copy)
                        nc.sync.dma_start(out=outT[mo * P:(mo + 1) * P, ssl],
                                          in_=ot)
                    for mo in range(KE):
                        psum_w = ps.tile([P, 512], F32, tag="p")
                        for k in range(3):
                            nc.tensor.matmul(psum_w,
                                             t_wo[:, k, mo * P:(mo + 1) * P],
                                             aoT[:, k, :],
                                             start=(k == 0), stop=False)
                        if mo == 1 and deferred is not None:
                            norm_head(*deferred)
                            deferred = None
                        if len(pend_w) >= 2:
                            fin_w(*pend_w.pop(0))
                        pend_w.append((psum_w, mo))
                    for pw in pend_w:
                        fin_w(*pw)

            stack.close()

    nc.compile()
    return nc


def _host_inputs(inputs):
    """Per-core input maps (host-side sharding + weight pre-tiling)."""
    x = inputs["x"]
    W_DQ = inputs["W_DQ"].astype(np.float32)
    W_UQ = inputs["W_UQ"].astype(np.float32)
    W_QR = inputs["W_QR"].astype(np.float32)
    W_DKV = inputs["W_DKV"].astype(np.float32)
    W_UK = inputs["W_UK"].astype(np.float32)
    W_KR = inputs["W_KR"].astype(np.float32)
    W_UV = inputs["W_UV"].astype(np.float32)
    W_O = inputs["W_O"].astype(np.float32)

    # fold W_DQ into the query up-projections (pure reparameterization)
    W_DQU = (W_DQ @ W_UQ) * SCALE                  # [E, E]
    W_DQR = (W_DQ @ W_QR) * SCALE                  # [E, R*H]

    wDKV_t = _rhs_layout(W_DKV).astype(NPBF16)
    wKR_t = _rhs_layout(_rope_perm_cols(W_KR)).astype(NPBF16)

    half = R // 2
    freqs = BASE ** (-np.arange(half, dtype=np.float64) / half)
    theta = np.arange(S, dtype=np.float64)[None, :] * freqs[:, None]   # [32, S]
    cs = np.concatenate([np.cos(theta), np.cos(theta),
                         -np.sin(theta), np.sin(theta)], 0).astype(NPBF16)
    p = np.arange(P)[:, None]
    f = np.arange(P)[None, :]
    maskadd = np.where(p <= f, 1.0, 0.0).astype(NPBF16)
    ones = np.ones((P, 1), NPBF16)
    onescol = np.ones((1, P), np.float32)

    shared = {
        "wDKV": wDKV_t, "wKR": wKR_t, "csq": cs, "maskin": maskadd,
        "ones_in": ones, "onescol_in": onescol,
    }
    gsets = []
    for g in range(4):
        cs0, ce0 = g * GCOL, (g + 1) * GCOL
        gsets.append({
            "wDQU": _rhs_layout(W_DQU[:, cs0:ce0]).astype(NPBF16),
            "wDQR": _rhs_layout(_rope_perm_cols(
                W_DQR[:, g * HPG * R:(g + 1) * HPG * R])).astype(NPBF16),
            "wUK": _rhs_layout(W_UK[:, cs0:ce0]).astype(NPBF16),
            "wUV": _rhs_layout(W_UV[:, cs0:ce0]).astype(NPBF16),
            "wO": _rhs_layout(W_O[cs0:ce0, :]).astype(NPBF16),
        })
    in_maps = []
    for c in range(8):
        b, g = divmod(c, 4)
        xT = np.ascontiguousarray(
            x[b].T.reshape(KE, P, S).transpose(1, 0, 2)).astype(NPBF16)
        m = {"xT": xT}
        m.update(shared)
        m.update(gsets[g])
        in_maps.append(m)
    return in_maps


def _assemble(results):
    out = np.empty((B, S, E), np.float32)
    for b in range(B):
        acc = results[4 * b]["outT"].astype(np.float32)
        for g in range(1, 4):
            acc = acc + results[4 * b + g]["outT"].astype(np.float32)
        out[b] = acc.T
    return out


def kernel(**inputs):
    inputs = {k: np.asarray(v) for k, v in inputs.items()}
    if "nc" not in _CACHE:
        _CACHE["nc"] = build_nc()
    nc = _CACHE["nc"]
    in_maps = _host_inputs(inputs)
    res = run_bass_kernel_spmd(nc, in_maps, core_ids=list(range(8)))
    return _assemble(res.results)


# revision 5
# speedup vs baseline: 1.0919x; 1.0040x over previous
"""Multi-Head Latent Attention for Trainium2, sharded over 8 NeuronCores.

Sharding: batch (2) x head-groups (4 of 4 heads each) -> 8 cores.

v3 design:
- W_DQ folded into W_UQ / W_QR on the host (weight reparameterization), so
  the device never computes c_Q.
- All matmuls bf16 (fp32 PSUM accumulate); end-to-end max-rel ~4e-3 vs
  float64 (gate 2e-2). Everything SBUF-resident between phases.
- A: q_C^T, q_R^T(rope), c_KV^T, k_rope^T from streamed 512-col x blocks.
  Rope is 3 fused DVE mul-from-psum ops + 1 Pool add (not 6 DVE ops).
- B: k_C^T, v_C from SBUF c_KV^T.
- C: causal attention. Diagonal key-tiles only compute the live query
  columns (N = 512-128r). Softmax denominator: off-diagonal exp tiles are
  pre-summed in groups of 4 on the Pool engine so the PE does 1 ones-matmul
  per group instead of 4. 1/den is broadcast across partitions with a K=1
  PE matmul instead of a DRAM bounce.
- D: W_O projection, bf16 partials to DRAM; host sums groups + transposes.
"""
import numpy as np
import ml_dtypes

import concourse.bass as bass
import concourse.mybir as mybir
import concourse.tile as tile
from concourse import bacc
from concourse.bass_utils import run_bass_kernel_spmd

F32 = mybir.dt.float32
F32R = mybir.dt.float32r
BF16 = mybir.dt.bfloat16
Exp = mybir.ActivationFunctionType.Exp
Mult = mybir.AluOpType.mult
Add = mybir.AluOpType.add
NPBF16 = ml_dtypes.bfloat16

B, S, E = 2, 2048, 2048
H = 16
DH = 128
LOW = 512
R = 64
BASE = 10000.0
HPG = 4               # heads per group (per core)
GCOL = HPG * DH       # 512 columns of this group's heads
P = 128
KE = E // P           # 16 k-tiles over E
KL = LOW // P         # 4 k-tiles over LOW
SBN = S // 512        # 4 seq blocks of 512
NEG = -3.0e38
SCALE = 1.0 / float(np.sqrt(DH + R))

_CACHE = {}


def _rhs_layout(w):
    """[K, N] -> [128, KT, N]: element [p, ko, n] = w[ko*128+p, n].
    Slicing [:, ko, m0:m0+128] is also the lhsT tile for (ko, m-block)."""
    K, N = w.shape
    return np.ascontiguousarray(w.reshape(K // P, P, N).transpose(1, 0, 2))


def _rope_perm_cols(w, rope_dim=R):
    """Permute each rope_dim-column block to [evens, odds] order."""
    K, M = w.shape
    nh = M // rope_dim
    w = w.reshape(K, nh, rope_dim)
    perm = np.concatenate([np.arange(0, rope_dim, 2), np.arange(1, rope_dim, 2)])
    return np.ascontiguousarray(w[:, :, perm].reshape(K, M))


def build_nc():
    nc = bacc.Bacc("TRN2", target_bir_lowering=False, debug=False, num_devices=8)

    xTd = nc.dram_tensor("xT", [P, KE, S], BF16, kind="ExternalInput")
    wDQU = nc.dram_tensor("wDQU", [P, KE, GCOL], BF16, kind="ExternalInput")
    wDQR = nc.dram_tensor("wDQR", [P, KE, HPG * R], BF16, kind="ExternalInput")
    wDKV = nc.dram_tensor("wDKV", [P, KE, LOW], BF16, kind="ExternalInput")
    wKR = nc.dram_tensor("wKR", [P, KE, R], BF16, kind="ExternalInput")
    wUK = nc.dram_tensor("wUK", [P, KL, GCOL], BF16, kind="ExternalInput")
    wUV = nc.dram_tensor("wUV", [P, KL, GCOL], BF16, kind="ExternalInput")
    wO = nc.dram_tensor("wO", [P, HPG, E], BF16, kind="ExternalInput")
    csq = nc.dram_tensor("csq", [P, S], BF16, kind="ExternalInput")
    maskin = nc.dram_tensor("maskin", [P, P], BF16, kind="ExternalInput")
    ones_in = nc.dram_tensor("ones_in", [P, 1], BF16, kind="ExternalInput")
    onescol_in = nc.dram_tensor("onescol_in", [1, P], F32R, kind="ExternalInput")

    outT = nc.dram_tensor("outT", [E, S], BF16, kind="ExternalOutput")

    with tile.TileContext(nc) as tc:
        with tc.tile_pool(name="persist", bufs=1) as persist:
            t_cs = persist.tile([P, S], BF16, tag="cs")
            t_mask = persist.tile([P, P], BF16, tag="mask")
            t_ones = persist.tile([P, 1], BF16, tag="ones")
            t_onescol = persist.tile([1, P], F32R, tag="onescol")
            qCT = persist.tile([P, HPG, S], BF16, tag="qCT")
            qrT = persist.tile([R, HPG, S], BF16, tag="qrT")
            kropeT = persist.tile([R, S], BF16, tag="kropeT")
            kCT = persist.tile([P, HPG, S], BF16, tag="kCT")
            vC = persist.tile([P, S // P, GCOL], BF16, tag="vC")
            ckvT = persist.tile([P, KL, S], BF16, tag="ckvT")
            t_wuk = persist.tile([P, KL, GCOL], BF16, tag="wuk")
            t_wuv = persist.tile([P, KL, GCOL], BF16, tag="wuv")
            t_wo = persist.tile([P, HPG, E], BF16, tag="wo")

            # one [128,512] fp32 psum pool spans all phases: A projections,
            # B up-projections, C scores + bcast, D W_O. Sharing the pool
            # (same tag) avoids bank-reuse barriers at phase boundaries.
            import contextlib
            stack = contextlib.ExitStack()
            ps = stack.enter_context(tc.tile_pool(name="ps", bufs=4,
                                                  space="PSUM"))

            # ---- Phase A: q_C^T, q_R^T(rope), c_KV^T, k_rope^T from x ----
            with (
                tc.tile_pool(name="aw", bufs=1) as awp,
                tc.tile_pool(name="xp", bufs=2) as xp,
                tc.tile_pool(name="ropew", bufs=1) as rp,
                tc.tile_pool(name="ps_kr", bufs=1, space="PSUM") as ps_kr,
            ):
                nc.sync.dma_start(out=t_ones, in_=ones_in[:, :])
                t_wdqu = awp.tile([P, KE, GCOL], BF16, tag="wdqu")
                xt0 = xp.tile([P, KE, 512], BF16, tag="xt")
                for q0, q1 in ((0, 2), (2, 4), (4, 6), (6, 8), (8, 12),
                               (12, KE)):
                    nc.sync.dma_start(out=t_wdqu[:, q0:q1, :],
                                      in_=wDQU[:, q0:q1, :])
                    nc.sync.dma_start(out=xt0[:, q0:q1, :],
                                      in_=xTd[:, q0:q1, 0:512])
                warm = awp.tile([P, 1], BF16, tag="warm")
                nc.scalar.activation(out=warm, in_=t_ones, func=Exp)
                t_wdqr = awp.tile([P, KE, HPG * R], BF16, tag="wdqr")
                nc.sync.dma_start(out=t_wdqr, in_=wDQR[:, :, :])
                nc.sync.dma_start(out=t_cs, in_=csq[:, :])
                t_wdkv = awp.tile([P, KE, LOW], BF16, tag="wdkv")
                nc.sync.dma_start(out=t_wdkv, in_=wDKV[:, :, :])
                t_wkr = awp.tile([P, KE, R], BF16, tag="wkr")
                nc.sync.dma_start(out=t_wkr, in_=wKR[:, :, :])
                nc.sync.dma_start(out=t_onescol, in_=onescol_in[:, :])
                nc.sync.dma_start(out=t_mask, in_=maskin[:, :])

                def rope_write(psum, base, dst, ssl):
                    """dst[0:64,:] = rope(psum[base:base+64, :]), 512 wide.
                    t1 = [x1;x2]*[c;c] on DVE (fused mul from psum),
                    t2 = [x2;x1]*[-s;s] on DVE (two 32-row fused muls),
                    dst = t1 + t2 on Pool."""
                    t1 = rp.tile([R, 512], F32, tag="t1")
                    nc.vector.tensor_tensor(t1, psum[base:base + R, :],
                                            t_cs[0:R, ssl], Mult)
                    t2 = rp.tile([R, 512], F32, tag="t2")
                    nc.vector.tensor_tensor(t2[0:32, :],
                                            psum[base + 32:base + R, :],
                                            t_cs[R:R + 32, ssl], Mult)
                    nc.vector.tensor_tensor(t2[32:R, :],
                                            psum[base:base + 32, :],
                                            t_cs[R + 32:2 * R, ssl], Mult)
                    nc.gpsimd.tensor_tensor(dst, t1, t2, Add)

                for sb in range(SBN):
                    ssl = slice(sb * 512, (sb + 1) * 512)
                    if sb == 0:
                        xt = xt0
                    else:
                        xt = xp.tile([P, KE, 512], BF16, tag="xt")
                        nc.sync.dma_start(out=xt, in_=xTd[:, :, ssl])
                    for mo in range(HPG):      # q_C^T
                        psum = ps.tile([P, 512], F32, tag="p")
                        for k in range(KE):
                            nc.tensor.matmul(
                                psum, t_wdqu[:, k, mo * P:(mo + 1) * P],
                                xt[:, k, :], start=(k == 0), stop=(k == KE - 1))
                        nc.vector.tensor_copy(out=qCT[:, mo, ssl], in_=psum)
                    for j in range(2):         # q_R^T (2 heads per psum)
                        psum = ps.tile([P, 512], F32, tag="p")
                        for k in range(KE):
                            nc.tensor.matmul(
                                psum, t_wdqr[:, k, j * P:(j + 1) * P],
                                xt[:, k, :], start=(k == 0), stop=(k == KE - 1))
                        rope_write(psum, 0, qrT[:, 2 * j, ssl], ssl)
                        rope_write(psum, R, qrT[:, 2 * j + 1, ssl], ssl)
                    for mo in range(KL):       # c_KV^T
                        psum = ps.tile([P, 512], F32, tag="p")
                        for k in range(KE):
                            nc.tensor.matmul(
                                psum, t_wdkv[:, k, mo * P:(mo + 1) * P],
                                xt[:, k, :], start=(k == 0), stop=(k == KE - 1))
                        nc.vector.tensor_copy(out=ckvT[:, mo, ssl], in_=psum)
                    psum = ps_kr.tile([R, 512], F32, tag="pkr")   # k_rope^T
                    for k in range(KE):
                        nc.tensor.matmul(psum, t_wkr[:, k, :], xt[:, k, :],
                                         start=(k == 0), stop=(k == KE - 1))
                    rope_write(psum, 0, kropeT[:, ssl], ssl)

            # ---- Phase B: k_C^T, v_C from c_KV^T ------------------------
            with (
                tc.tile_pool(name="ps_o", bufs=2, space="PSUM") as ps_o,
                tc.tile_pool(name="ps_d", bufs=2, space="PSUM") as ps_d,
            ):
                nc.sync.dma_start(out=t_wuk, in_=wUK[:, :, :])
                nc.sync.dma_start(out=t_wuv, in_=wUV[:, :, :])
                nc.sync.dma_start(out=t_wo, in_=wO[:, :, :])
                for sb in range(SBN):
                    ssl = slice(sb * 512, (sb + 1) * 512)
                    for h in range(HPG):
                        psum = ps.tile([P, 512], F32, tag="p")
                        for k in range(KL):
                            nc.tensor.matmul(
                                psum, t_wuk[:, k, h * P:(h + 1) * P],
                                ckvT[:, k, ssl], start=(k == 0),
                                stop=(k == KL - 1))
                        nc.vector.tensor_copy(out=kCT[:, h, ssl], in_=psum)
                    for loc in range(4):
                        st = sb * 4 + loc
                        psum = ps.tile([P, GCOL], F32, tag="p")
                        for k in range(KL):
                            nc.tensor.matmul(
                                psum, ckvT[:, k, st * P:(st + 1) * P],
                                t_wuv[:, k, :], start=(k == 0),
                                stop=(k == KL - 1))
                        nc.vector.tensor_copy(out=vC[:, st, :], in_=psum)

                # ---- Phase C: attention;  Phase D: W_O ------------------
                att = stack.enter_context(tc.tile_pool(name="att", bufs=8))
                accp = stack.enter_context(tc.tile_pool(name="accp", bufs=2))
                bcp = stack.enter_context(tc.tile_pool(name="bcp", bufs=2))
                rcp = stack.enter_context(tc.tile_pool(name="rcp", bufs=2))
                aop = stack.enter_context(tc.tile_pool(name="aop", bufs=2))
                oout = stack.enter_context(tc.tile_pool(name="oout", bufs=8))
                def norm_head(aoT, h, aou, recip):
                    """Deferred 1/den broadcast + aoT[h] = aou * bc."""
                    psum_b = ps.tile([P, 512], F32, tag="p")
                    nc.tensor.matmul(psum_b, t_onescol, recip,
                                     start=True, stop=True)
                    nc.vector.tensor_tensor(aoT[:, h, :], aou, psum_b, Mult)

                for sb in range(SBN):
                    ssl = slice(sb * 512, (sb + 1) * 512)
                    aoT = aop.tile([P, HPG, 512], BF16, tag="aoT")
                    T = 4 * (sb + 1)
                    deferred = None
                    for h in range(HPG):
                        psum_o = ps_o.tile([P, 512], F32, tag="p")
                        psum_d = ps_d.tile([1, 512], F32, tag="p")
                        pend_den = None     # (acc, start) awaiting PE issue
                        # off-diagonal key tiles, denominator batched by 4
                        for g in range(sb):
                            exps = []
                            for j in range(4):
                                tt = 4 * g + j
                                tsl = slice(tt * P, (tt + 1) * P)
                                psum_s = ps.tile([P, 512], F32, tag="p")
                                nc.tensor.matmul(psum_s, kCT[:, h, tsl],
                                                 qCT[:, h, ssl],
                                                 start=True, stop=False)
                                nc.tensor.matmul(psum_s, kropeT[:, tsl],
                                                 qrT[:, h, ssl],
                                                 start=False, stop=True)
                                expT = att.tile([P, 512], BF16, tag="expT")
                                nc.scalar.activation(out=expT, in_=psum_s,
                                                     func=Exp)
                                nc.tensor.matmul(
                                    psum_o, vC[:, tt, h * DH:(h + 1) * DH],
                                    expT, start=(tt == 0), stop=False)
                                exps.append(expT)
                            if pend_den is not None:
                                nc.tensor.matmul(psum_d, t_ones, pend_den[0],
                                                 start=pend_den[1], stop=False)
                            acc = accp.tile([P, 512], BF16, tag="acc")
                            nc.vector.tensor_tensor(acc, exps[0], exps[1], Add)
                            nc.vector.tensor_tensor(acc, acc, exps[2], Add)
                            nc.vector.tensor_tensor(acc, acc, exps[3], Add)
                            pend_den = (acc, g == 0)
                            if g == 0 and deferred is not None:
                                norm_head(*deferred)
                                deferred = None
                        # diagonal key tiles, trimmed to live queries
                        for r in range(4):
                            tt = 4 * sb + r
                            tsl = slice(tt * P, (tt + 1) * P)
                            npr = 512 - P * r
                            qsl = slice(sb * 512 + P * r, (sb + 1) * 512)
                            psum_s = ps.tile([P, 512], F32, tag="p")
                            nc.tensor.matmul(psum_s[:, 0:npr],
                                             kCT[:, h, tsl], qCT[:, h, qsl],
                                             start=True, stop=False)
                            nc.tensor.matmul(psum_s[:, 0:npr],
                                             kropeT[:, tsl], qrT[:, h, qsl],
                                             start=False, stop=True)
                            expT = att.tile([P, 512], BF16, tag="expT")
                            nc.scalar.activation(out=expT[:, 0:npr],
                                                 in_=psum_s[:, 0:npr],
                                                 func=Exp)
                            nc.gpsimd.tensor_tensor(expT[:, 0:P],
                                                    expT[:, 0:P],
                                                    t_mask, Mult)
                            first = (sb == 0 and r == 0)
                            nc.tensor.matmul(
                                psum_o[:, P * r:512],
                                vC[:, tt, h * DH:(h + 1) * DH],
                                expT[:, 0:npr], start=first,
                                stop=(r == 3))
                            if r == 0 and pend_den is not None:
                                nc.tensor.matmul(psum_d, t_ones, pend_den[0],
                                                 start=pend_den[1], stop=False)
                                pend_den = None
                            nc.tensor.matmul(
                                psum_d[0:1, P * r:512], t_ones,
                                expT[:, 0:npr], start=first,
                                stop=(r == 3))
                            if r == 1 and deferred is not None:
                                norm_head(*deferred)
                                deferred = None
                        aou = bcp.tile([P, 512], BF16, tag="aou")
                        nc.scalar.activation(
                            out=aou, in_=psum_o,
                            func=mybir.ActivationFunctionType.Copy)
                        recip = rcp.tile([1, 512], F32R, tag="recip")
                        with nc.allow_low_precision(
                                reason="f32r is 32-bit storage; fp32 recip"):
                            nc.vector.reciprocal(out=recip, in_=psum_d)
                        deferred = (aoT, h, aou, recip)
                    # W_O: contract heads 0..2 immediately; defer each
                    # group's final h3 step one group so the last head's
                    # normalization chain is hidden behind real work.
                    pend_w = []         # [(psum_w, mo)], flushed 2 behind
                    def fin_w(psum_w, mo):
                        nc.tensor.matmul(psum_w,
                                         t_wo[:, 3, mo * P:(mo + 1) * P],
                                         aoT[:, 3, :], start=False, stop=True)
                        ot = oout.tile([P, 512], BF16, tag="oout")
                        if mo % 2 == 0:
                            nc.vector.tensor_copy(out=ot, in_=psum_w)
                        else:
                            nc.scalar.activation(out=ot, in_=psum_w,
                                                 func=mybir.ActivationFunctionType.Copy)
                        nc.sync.dma_start(out=outT[mo * P:(mo + 1) * P, ssl],
                                          in_=ot)
                    for mo in range(KE):
                        psum_w = ps.tile([P, 512], F32, tag="p")
                        for k in range(3):
                            nc.tensor.matmul(psum_w,
                                             t_wo[:, k, mo * P:(mo + 1) * P],
                                             aoT[:, k, :],
                                             start=(k == 0), stop=False)
                        if mo == 1 and deferred is not None:
                            norm_head(*deferred)
                            deferred = None
                        if len(pend_w) >= 2:
                            fin_w(*pend_w.pop(0))
                        pend_w.append((psum_w, mo))
                    for pw in pend_w:
                        fin_w(*pw)

            stack.close()

    nc.compile()
    return nc


def _host_inputs(inputs):
    """Per-core input maps (host-side sharding + weight pre-tiling)."""
    x = inputs["x"]
    W_DQ = inputs["W_DQ"].astype(np.float32)
    W_UQ = inputs["W_UQ"].astype(np.float32)
    W_QR = inputs["W_QR"].astype(np.float32)
    W_DKV = inputs["W_DKV"].astype(np.float32)
    W_UK = inputs["W_UK"].astype(np.float32)
    W_KR = inputs["W_KR"].astype(np.float32)
    W_UV = inputs["W_UV"].astype(np.float32)
    W_O = inputs["W_O"].astype(np.float32)

    # fold W_DQ into the query up-projections (pure reparameterization)
    W_DQU = (W_DQ @ W_UQ) * SCALE                  # [E, E]
    W_DQR = (W_DQ @ W_QR) * SCALE                  # [E, R*H]

    wDKV_t = _rhs_layout(W_DKV).astype(NPBF16)
    wKR_t = _rhs_layout(_rope_perm_cols(W_KR)).astype(NPBF16)

    half = R // 2
    freqs = BASE ** (-np.arange(half, dtype=np.float64) / half)
    theta = np.arange(S, dtype=np.float64)[None, :] * freqs[:, None]   # [32, S]
    cs = np.concatenate([np.cos(theta), np.cos(theta),
                         -np.sin(theta), np.sin(theta)], 0).astype(NPBF16)
    p = np.arange(P)[:, None]
    f = np.arange(P)[None, :]
    maskadd = np.where(p <= f, 1.0, 0.0).astype(NPBF16)
    ones = np.ones((P, 1), NPBF16)
    onescol = np.ones((1, P), np.float32)

    shared = {
        "wDKV": wDKV_t, "wKR": wKR_t, "csq": cs, "maskin": maskadd,
        "ones_in": ones, "onescol_in": onescol,
    }
    gsets = []
    for g in range(4):
        cs0, ce0 = g * GCOL, (g + 1) * GCOL
        gsets.append({
            "wDQU": _rhs_layout(W_DQU[:, cs0:ce0]).astype(NPBF16),
            "wDQR": _rhs_layout(_rope_perm_cols(
                W_DQR[:, g * HPG * R:(g + 1) * HPG * R])).astype(NPBF16),
            "wUK": _rhs_layout(W_UK[:, cs0:ce0]).astype(NPBF16),
            "wUV": _rhs_layout(W_UV[:, cs0:ce0]).astype(NPBF16),
            "wO": _rhs_layout(W_O[cs0:ce0, :]).astype(NPBF16),
        })
    in_maps = []
    for c in range(8):
        b, g = divmod(c, 4)
        xT = np.ascontiguousarray(
            x[b].T.reshape(KE, P, S).transpose(1, 0, 2)).astype(NPBF16)
        m = {"xT": xT}
        m.update(shared)
        m.update(gsets[g])
        in_maps.append(m)
    return in_maps


def _assemble(results):
    out = np.empty((B, S, E), np.float32)
    for b in range(B):
        acc = results[4 * b]["outT"].astype(np.float32)
        for g in range(1, 4):
            acc = acc + results[4 * b + g]["outT"].astype(np.float32)
        out[b] = acc.T
    return out


def kernel(**inputs):
    inputs = {k: np.asarray(v) for k, v in inputs.items()}
    if "nc" not in _CACHE:
        _CACHE["nc"] = build_nc()
    nc = _CACHE["nc"]
    in_maps = _host_inputs(inputs)
    res = run_bass_kernel_spmd(nc, in_maps, core_ids=list(range(8)))
    return _assemble(res.results)
